# revision 2
# baseline (speedup 1.0000x reference)
"""3-layer GCN (GCNConv x3 + leaky_relu + first-node-per-graph readout) on
8 Trainium2 NeuronCores via Bass/Tile.

Strategy (readout-driven pruning + balanced per-edge streams):
  - Readout keeps only each graph's first node, so layer-3 needs z for ~1.5k
    nodes (D2 = srcs of first-node edges), layer-2 output only for D2, and
    layer-1 output only for S2 = srcs of edges into D2.  Only edges into S2
    (~228k of 320k) are processed at layer 1, edges into D2 (~25k) at layer
    2.  Exact, not an approximation.
  - Node->core/window ownership is load-balanced host-side (degree-sorted
    snake deal over cores x windows), so cross-core max-chunk padding is
    ~zero: L1 is 224 chunks/core of 128 edges (vs 291 for the naive
    node//2500 split).
  - L1 aggregation is per-edge: host packs fp8(dis*x)[src] rows of each
    64-dst inner window into 128-slot chunks (large sequential DMAs, no
    gather); one fp8 matmul per chunk (lhsT = source rows, rhs = slot->dst
    one-hot loaded once as a constant) accumulates into a [128,64] PSUM
    window at ~2x the op rate of 128-wide windows.  GCN normalization is
    factored: tables store dis*h, dis^2(dst) rides a DVE multiply, the bias
    dis (x) b is a rank-1 matmul, and leaky-relu is one fused ACT op.
  - The layer-1 table is exchanged with FOUR AllGathers issued as their
    window ranges complete, overlapping the collectives with the remaining
    L1 compute; layer-2 dma_gathers (<=1024 idx/call) start as soon as
    their split's exchange lands.
  - L3 aggregates on the z-owner side via host-packed per-graph count
    matrices (2 matmuls) and a single tiny [128,32] f32 AllReduce; no z
    table, no AllGather, no gather.

kernel(**inputs) takes the full unsharded inputs and returns the full
[n_graphs, 32] float32 output.  Measured ~2.5x faster than the quota-8
grouped-stream baseline on the repeat-marginal metric; rel err ~9.1e-3.
"""

import sys

sys.path.insert(0, "/opt/trn_rl_repo")

import numpy as np

import concourse.bacc as bacc
import concourse.mybir as mybir
import concourse.tile as tile
from concourse.bass_utils import run_bass_kernel_spmd

F32 = mybir.dt.float32
BF16 = mybir.dt.bfloat16
I16 = mybir.dt.int16
FP8 = mybir.dt.float8e4

N_CORES = 8
C0, C1, C2, C3 = 128, 256, 256, 32
DW = 64  # inner-window width (dsts per aggregation PSUM)
EC = 64  # layer-1 chunks per estream DMA load
GC = 8  # chunks per dma_gather call (HW limit: 1024 indices)

# ---------------------------------------------------------------------------
# Host-side prep
# ---------------------------------------------------------------------------


def _snake_bins(items, wts, nbins):
    """Deal items (sorted by weight desc) into nbins boustrophedon; returns
    list of index-arrays. Balances both count (+-1) and weight sum."""
    order = np.argsort(-wts, kind="stable")
    bins = [[] for _ in range(nbins)]
    for t, j in enumerate(order):
        r, c = divmod(t, nbins)
        if r % 2:
            c = nbins - 1 - c
        bins[c].append(items[j])
    return [np.array(b, dtype=np.int64) for b in bins]


def _pack_gather_idx(idx, n_slots):
    """int32 row indices -> dma_gather int16 layout [128, n_slots//16]."""
    assert n_slots % 16 == 0
    a = np.zeros(n_slots, np.int16)
    a[: len(idx)] = idx.astype(np.int16)
    a = a.reshape(n_slots // 16, 16).T  # [16, cols]
    return np.tile(a, (8, 1))  # [128, cols]


def host_prep(x, src, dst, batch, W1, b1, W2, b2, W3, b3, n_graphs):
    import ml_dtypes

    N = x.shape[0]
    G = int(n_graphs)

    deg = np.bincount(dst, minlength=N).astype(np.float32)
    dis = np.where(deg > 0, 1.0 / np.sqrt(np.maximum(deg, 1.0)), 0.0).astype(
        np.float32
    )

    first = np.full(G, N, np.int64)
    np.minimum.at(first, batch.astype(np.int64), np.arange(N))

    is_first = np.zeros(N, bool)
    is_first[first] = True
    gid_of = np.full(N, -1, np.int64)
    gid_of[first] = np.arange(G)

    e3 = np.nonzero(is_first[dst])[0]
    D2 = np.unique(src[e3]).astype(np.int64)
    in_d2 = np.zeros(N, bool)
    in_d2[D2] = True
    e2 = np.nonzero(in_d2[dst])[0]
    S2 = np.unique(src[e2]).astype(np.int64)
    in_s2 = np.zeros(N, bool)
    in_s2[S2] = True
    e1 = np.nonzero(in_s2[dst])[0]

    # --- balanced ownership: S2 -> (core, inner-window of DW, slot) ---
    IWC = int(-(-len(S2) // (N_CORES * DW)))  # inner windows per core
    if IWC % 2:
        IWC += 1  # outer windows pair two inner windows
    W1C = IWC // 2  # outer 128-row windows per core
    deg1 = np.bincount(dst[e1], minlength=N)[S2].astype(np.float32)
    bins1 = _snake_bins(S2, deg1, N_CORES * IWC)
    # bin c -> (window = c // N_CORES, core = c % N_CORES): consecutive snake
    # bins (similar weight) land on the 8 cores of the same window index.
    s2core = np.full(N, -1, np.int64)
    s2row = np.full(N, -1, np.int64)  # row in core's [IWC*DW] h1 layout
    for c, nodes in enumerate(bins1):
        w, i = c // N_CORES, c % N_CORES
        if len(nodes):
            s2core[nodes] = i
            s2row[nodes] = w * DW + np.arange(len(nodes))

    # --- balanced ownership: D2 -> (core, window2 of 128, slot) ---
    W2C = int(-(-len(D2) // (N_CORES * 128)))
    deg2 = np.bincount(dst[e2], minlength=N)[D2].astype(np.float32)
    bins2 = _snake_bins(D2, deg2, N_CORES * W2C)
    d2core = np.full(N, -1, np.int64)
    d2row = np.full(N, -1, np.int64)
    for c, nodes in enumerate(bins2):
        w, i = c // N_CORES, c % N_CORES
        if len(nodes):
            d2core[nodes] = i
            d2row[nodes] = w * 128 + np.arange(len(nodes))

    # --- L1 per-edge streams ---
    ecore = s2core[dst[e1]]
    eiw = s2row[dst[e1]] // DW
    eslot = s2row[dst[e1]] % DW
    cnt1 = np.zeros((N_CORES, IWC), np.int64)
    np.add.at(cnt1, (ecore, eiw), 1)
    P1 = np.maximum(1, -(-cnt1.max(axis=0) // 128))  # chunks per inner window
    NC1 = int(P1.sum())
    cbase1 = np.concatenate([[0], np.cumsum(P1)]).astype(int)

    xt_bf16 = (dis[:, None] * x).astype(ml_dtypes.bfloat16)

    estreams, sm2s = [], []
    for i in range(N_CORES):
        idxs = np.full(NC1 * 128, -1, np.int64)
        scol = np.full(NC1 * 128, -1, np.int64)
        for w in range(IWC):
            m = (ecore == i) & (eiw == w)
            ee = e1[m]
            order = np.argsort(eslot[m], kind="stable")
            n = len(ee)
            base = cbase1[w] * 128
            idxs[base : base + n] = src[ee][order]
            scol[base : base + n] = eslot[m][order]
        est = xt_bf16[np.maximum(idxs, 0)].astype(ml_dtypes.float8_e4m3)
        est[idxs < 0] = 0
        estreams.append(
            np.ascontiguousarray(est.reshape(NC1, 128, C0).transpose(1, 0, 2))
        )
        sm = np.zeros((NC1, 128, DW), np.float32)
        kj, rj = np.divmod(np.nonzero(scol >= 0)[0], 128)
        sm[kj, rj, scol[scol >= 0]] = 1.0
        sm2s.append(
            np.ascontiguousarray(
                sm.transpose(1, 0, 2).reshape(128, NC1 * DW)
            ).astype(ml_dtypes.float8_e4m3)
        )

    # --- L2 edge streams (gather h1 rows; dst in balanced D2 layout) ---
    # h1 exchange is split into NSPL AllGathers over outer-window ranges so
    # transfers overlap L1 compute; only the last split's exchange is exposed.
    NSPL = 4
    bnd = np.linspace(0, W1C, NSPL + 1).round().astype(int)
    bounds = tuple(int(b) for b in bnd)
    aw = s2row[src[e2]] // 128  # outer window of source
    spl = np.searchsorted(bnd[1:], aw, side="right")  # split of source row
    nwq = bnd[1:] - bnd[:-1]
    h1row = (
        s2core[src[e2]] * (nwq[spl] * 128)
        + s2row[src[e2]]
        - bnd[spl] * 128
    )
    o2 = d2core[dst[e2]]
    pos2 = d2row[dst[e2]]
    w2 = pos2 // 128
    slot2 = pos2 % 128

    def l2_streams(mask):
        cnt = np.zeros((N_CORES, W2C), np.int64)
        np.add.at(cnt, (o2[mask], w2[mask]), 1)
        P = np.maximum(1, -(-cnt.max(axis=0) // 128))
        NCk = int(P.sum())
        cb = np.concatenate([[0], np.cumsum(P)]).astype(int)
        idx_s, slot_s = [], []
        for i in range(N_CORES):
            idxs = np.zeros(NCk * 128, np.int64)
            slots = np.full(NCk * 128, -1.0, np.float32)
            for w in range(W2C):
                m = mask & (o2 == i) & (w2 == w)
                n = int(m.sum())
                base = cb[w] * 128
                idxs[base : base + n] = h1row[m]
                slots[base : base + n] = slot2[m]
            idx_s.append(idxs)
            slot_s.append(slots)
        return P, NCk, idx_s, slot_s

    l2q = [l2_streams(spl == q) for q in range(NSPL)]

    # --- L3: per-zslot -> graph count matrices (aggregated on z owner) ---
    zsl = d2row[src[e3]]  # z row in owner's [W2C*128] layout
    zco = d2core[src[e3]]
    gg = gid_of[dst[e3]]

    # --- constants ---
    w1 = np.ascontiguousarray(W1).astype(ml_dtypes.bfloat16)  # [128, 256]
    w2r = np.ascontiguousarray(
        np.concatenate([W2[0:128, :], W2[128:256, :]], axis=1)
    ).astype(ml_dtypes.bfloat16)  # [128, 512]
    w3r = np.ascontiguousarray(
        np.concatenate([W3[0:128, :], W3[128:256, :]], axis=1)
    ).astype(ml_dtypes.bfloat16)  # [128, 64]
    b1r = b1.reshape(1, C1).astype(ml_dtypes.bfloat16)
    b2r = b2.reshape(1, C2).astype(ml_dtypes.bfloat16)
    b3p = np.zeros((128, C3), np.float32)
    b3p[:, :] = b3[None, :]
    disf = np.zeros((128, 1), np.float32)
    disf[:G, 0] = dis[first]
    iotaf = np.tile(
        np.arange(128, dtype=np.float32)[None, :], (128, 1)
    ).astype(ml_dtypes.bfloat16)

    in_maps = []
    for i in range(N_CORES):
        # dis of this core's S2 rows in h1 layout (padded to W1C*128)
        dloc = np.zeros(W1C * 128, np.float32)
        m = s2core == i
        dloc[s2row[m]] = dis[np.nonzero(m)[0]]
        dcb2 = np.tile((dloc * dloc)[None, :], (128, 1)).astype(np.float32)
        disrow = dloc.reshape(1, -1).astype(ml_dtypes.bfloat16)

        dloc2 = np.zeros(W2C * 128, np.float32)
        m2 = d2core == i
        dloc2[d2row[m2]] = dis[np.nonzero(m2)[0]]
        dd = (dloc2 * dloc2).reshape(W2C, 128)
        dcb2l2 = np.tile(
            np.concatenate([dd, dd], axis=1).reshape(1, -1), (128, 1)
        ).astype(np.float32)
        d2row_b = dloc2.reshape(1, -1).astype(ml_dtypes.bfloat16)

        s3 = np.zeros((W2C, 128, 128), np.float32)
        m3 = zco == i
        np.add.at(s3, (zsl[m3] // 128, zsl[m3] % 128, gg[m3]), 1.0)
        s3 = np.ascontiguousarray(
            s3.transpose(1, 0, 2).reshape(128, W2C * 128)
        ).astype(ml_dtypes.float8_e4m3)

        im = {}
        for q, (Pq, NCq, idx_s, slot_s) in enumerate(l2q):
            im[f"idx2q{q}"] = _pack_gather_idx(idx_s[i], NCq * 128)
            im[f"slot2q{q}"] = slot_s[i].reshape(NCq, 128).T.copy()
        in_maps.append(
            {
                **im,
                "estream": estreams[i],
                "sm2": sm2s[i],
                "s3": s3,
                "w1": w1,
                "w2r": w2r,
                "w3r": w3r,
                "b1r": b1r,
                "b2r": b2r,
                "b3p": b3p,
                "disrow": disrow,
                "d2row": d2row_b,
                "dcb2": dcb2,
                "dcb2l2": dcb2l2,
                "disf": disf,
                "iotaf": iotaf,
            }
        )

    meta = dict(
        N=N,
        G=G,
        W1C=W1C,
        IWC=IWC,
        W2C=W2C,
        NSPL=NSPL,
        bounds=bounds,
        P1=tuple(int(p) for p in P1),
        P2Q=tuple(tuple(int(p) for p in l2q[q][0]) for q in range(NSPL)),
        NC2Q=tuple(int(l2q[q][1]) for q in range(NSPL)),
        NC1=NC1,
    )
    return in_maps, meta


# ---------------------------------------------------------------------------
# Device program
# ---------------------------------------------------------------------------


def build_program(meta, compile_=True, repeat=1, parts="full"):
    W1C, IWC, W2C = meta["W1C"], meta["IWC"], meta["W2C"]
    P1 = meta["P1"]
    NC1 = meta["NC1"]
    NSPL = meta["NSPL"]
    bounds = meta["bounds"]
    nwq = [bounds[q + 1] - bounds[q] for q in range(NSPL)]
    P2Q = meta["P2Q"]
    NC2Q = meta["NC2Q"]

    nc = bacc.Bacc(
        "TRN2", target_bir_lowering=False, debug=False, num_devices=N_CORES
    )
    dp = nc.declare_dram_parameter
    estream_d = dp("estream", [128, NC1, C0], FP8, isOutput=False)
    sm2_d = dp("sm2", [128, NC1 * DW], FP8, isOutput=False)
    idx2q_d = [
        dp(f"idx2q{q}", [128, NC2Q[q] * 8], I16, isOutput=False)
        for q in range(NSPL)
    ]
    slot2q_d = [
        dp(f"slot2q{q}", [128, NC2Q[q]], F32, isOutput=False)
        for q in range(NSPL)
    ]
    s3_d = dp("s3", [128, W2C * 128], FP8, isOutput=False)
    w1_d = dp("w1", [128, C1], BF16, isOutput=False)
    w2r_d = dp("w2r", [128, 2 * C2], BF16, isOutput=False)
    w3r_d = dp("w3r", [128, 2 * C3], BF16, isOutput=False)
    b1r_d = dp("b1r", [1, C1], BF16, isOutput=False)
    b2r_d = dp("b2r", [1, C2], BF16, isOutput=False)
    b3p_d = dp("b3p", [128, C3], F32, isOutput=False)
    disrow_d = dp("disrow", [1, W1C * 128], BF16, isOutput=False)
    d2row_d = dp("d2row", [1, W2C * 128], BF16, isOutput=False)
    dcb2_d = dp("dcb2", [128, W1C * 128], F32, isOutput=False)
    dcb2l2_d = dp("dcb2l2", [128, W2C * 256], F32, isOutput=False)
    disf_d = dp("disf", [128, 1], F32, isOutput=False)
    iotaf_d = dp("iotaf", [128, 128], BF16, isOutput=False)
    out_d = dp("out", [128, C3], F32, isOutput=True)

    rg = [list(range(N_CORES))]
    AL = mybir.AluOpType
    ACT = mybir.ActivationFunctionType

    cstart1 = np.concatenate([[0], np.cumsum(P1)]).astype(int)
    cstart2q = [
        np.concatenate([[0], np.cumsum(P2Q[q])]).astype(int)
        for q in range(NSPL)
    ]
    maxP2 = max(max(P2Q[q]) for q in range(NSPL))
    calls1 = [(a, min(a + EC, NC1)) for a in range(0, NC1, EC)]
    calls2q = [
        [(a, min(a + GC, NC2Q[q])) for a in range(0, NC2Q[q], GC)]
        for q in range(NSPL)
    ]

    with tile.TileContext(nc) as tc:
        with (
            tc.tile_pool(name="const", bufs=1) as cpool,
            tc.tile_pool(name="work", bufs=4) as pool,
            tc.tile_pool(name="gath1", bufs=2) as gpool1,
            tc.tile_pool(name="gath2", bufs=3) as gpool2,
            tc.tile_pool(name="psum", bufs=2, space="PSUM") as psum,
            tc.tile_pool(name="psum2", bufs=1, space="PSUM") as psum2,
            tc.tile_pool(name="dram", bufs=1, space="DRAM") as dram,
        ):
            # ---- constants ----
            def cload(name, shape, dt, src_ap):
                t = cpool.tile(shape, dt, tag=name)
                nc.sync.dma_start(out=t[:], in_=src_ap)
                return t

            w1 = cload("w1", [128, C1], BF16, w1_d[:, :])
            w2r = cload("w2r", [128, 2 * C2], BF16, w2r_d[:, :])
            w3r = cload("w3r", [128, 2 * C3], BF16, w3r_d[:, :])
            b1r = cload("b1r", [1, C1], BF16, b1r_d[:, :])
            b2r = cload("b2r", [1, C2], BF16, b2r_d[:, :])
            b3p = cload("b3p", [128, C3], F32, b3p_d[:, :])
            disrow = cload("disrow", [1, W1C * 128], BF16, disrow_d[:, :])
            d2row = cload("d2row", [1, W2C * 128], BF16, d2row_d[:, :])
            dcb2 = cload("dcb2", [128, W1C * 128], F32, dcb2_d[:, :])
            dcb2l2 = cload("dcb2l2", [128, W2C * 256], F32, dcb2l2_d[:, :])
            disf = cload("disf", [128, 1], F32, disf_d[:, :])
            iotaf = cload("iotaf", [128, 128], BF16, iotaf_d[:, :])
            sm2 = cload("sm2", [128, NC1 * DW], FP8, sm2_d[:, :])
            s3c = cload("s3", [128, W2C * 128], FP8, s3_d[:, :])
            slot2q = [
                cload(f"slot2q{q}", [128, NC2Q[q]], F32, slot2q_d[q][:, :])
                for q in range(NSPL)
            ]

            def gather_calls(calls, idx_d, table, Cin, dt, tag, gp, tilechunks):
                out = []
                for a, b in calls:
                    cc = b - a
                    it = pool.tile([128, tilechunks * 8], I16, tag=f"{tag}i")
                    nc.sync.dma_start(
                        out=it[:, 0 : cc * 8], in_=idx_d[:, a * 8 : b * 8]
                    )
                    g = gp.tile([128, tilechunks, Cin], dt, tag=f"{tag}g")
                    nc.gpsimd.dma_gather(
                        g[:, 0:cc, :],
                        table[:, :],
                        it[:, 0 : cc * 8],
                        num_idxs=cc * 128,
                        num_idxs_reg=cc * 128,
                        elem_size=Cin,
                    )
                    out.append((a, g))
                return out

            def chunk_view(gts, c):
                for a, g in reversed(gts):
                    if c >= a:
                        return g[:, c - a, :]
                raise AssertionError

            def build_S(nchunks, slot_sb, c0, tag, tilechunks, dt=BF16):
                S = pool.tile([128, tilechunks * 128], dt, tag=tag)
                for j in range(nchunks):
                    nc.vector.tensor_scalar(
                        S[:, j * 128 : (j + 1) * 128],
                        iotaf[:],
                        slot_sb[:, c0 + j : c0 + j + 1],
                        None,
                        AL.is_equal,
                    )
                return S

            for _rep in range(repeat):
                h1q_in, h1q_full = [], []
                for q in range(NSPL):
                    h1q_in.append(
                        dram.tile(
                            [nwq[q] * 128, C1], FP8, tag=f"h1i{q}",
                            name=f"h1i{q}",
                        )
                    )
                    h1q_full.append(
                        dram.tile(
                            [N_CORES * nwq[q] * 128, C1],
                            FP8,
                            addr_space="Shared",
                            tag=f"h1f{q}",
                            name=f"h1f{q}",
                        )
                    )
                ar_in = dram.tile([128, C3], F32)
                ar_out = dram.tile([128, C3], F32, addr_space="Shared")

                # ================= layer 1 =================
                ets = []
                for ki, (a, b) in enumerate(calls1):
                    cc = b - a
                    et = gpool1.tile([128, EC, C0], FP8, tag="e1")
                    eng = nc.sync if ki % 2 == 0 else nc.scalar
                    eng.dma_start(out=et[:, 0:cc, :], in_=estream_d[:, a:b, :])
                    ets.append((a, et))
                for w in range(W1C):
                    if parts == "l1load":
                        continue
                    aggs = pool.tile([128, 128], BF16, tag="aggs1")
                    for h in range(2):
                        iw = 2 * w + h
                        a = cstart1[iw]
                        nch = P1[iw]
                        aggp = psum.tile([128, DW], F32, tag="aggp")
                        for j in range(nch):
                            nc.tensor.matmul(
                                aggp[:, :],
                                lhsT=chunk_view(ets, a + j),
                                rhs=sm2[:, (a + j) * DW : (a + j + 1) * DW],
                                start=(j == 0),
                                stop=(j == nch - 1),
                            )
                        nc.vector.tensor_tensor(
                            aggs[:, h * DW : (h + 1) * DW],
                            aggp[:, :],
                            dcb2[:, w * 128 + h * DW : w * 128 + (h + 1) * DW],
                            op=AL.mult,
                        )
                    if parts == "l1agg":
                        continue
                    h1p = psum.tile([128, C1], F32, tag="h1p")
                    nc.tensor.matmul(
                        h1p[:], lhsT=aggs[:], rhs=w1[:], start=True, stop=False
                    )
                    nc.tensor.matmul(
                        h1p[:],
                        lhsT=disrow[0:1, w * 128 : (w + 1) * 128],
                        rhs=b1r[0:1, :],
                        start=False,
                        stop=True,
                    )
                    t1 = pool.tile([128, C1], FP8, tag="t1")
                    nc.scalar.activation(t1[:], h1p[:], ACT.Lrelu, alpha=0.01)
                    q = int(np.searchsorted(bounds[1:], w, side="right"))
                    wb = w - bounds[q]
                    nc.sync.dma_start(
                        out=h1q_in[q][wb * 128 : (wb + 1) * 128, :], in_=t1[:]
                    )
                    if w == bounds[q + 1] - 1 and parts == "full":
                        nc.gpsimd.collective_compute(
                            "AllGather",
                            AL.bypass,
                            replica_groups=rg,
                            ins=[h1q_in[q].opt()],
                            outs=[h1q_full[q].opt()],
                        )
                if parts in ("l1", "l1load", "l1agg"):
                    continue

                # ================= layer 2 =================
                gts2q = [
                    gather_calls(
                        calls2q[q], idx2q_d[q], h1q_full[q], C1, FP8,
                        f"l2q{q}", gpool2, GC,
                    )
                    for q in range(NSPL)
                ]
                zts = []
                for w in range(W2C):
                    Sq = [
                        build_S(
                            cstart2q[q][w + 1] - cstart2q[q][w],
                            slot2q[q],
                            cstart2q[q][w],
                            f"S2q{q}",
                            maxP2,
                            dt=FP8,
                        )
                        for q in range(NSPL)
                    ]
                    nch = sum(
                        cstart2q[q][w + 1] - cstart2q[q][w] for q in range(NSPL)
                    )
                    aggp = psum2.tile([128, C1], F32, tag="l2p")
                    for h in range(2):
                        jglob = 0
                        for q in range(NSPL):
                            qa, qb = cstart2q[q][w], cstart2q[q][w + 1]
                            for j in range(qb - qa):
                                g = chunk_view(gts2q[q], qa + j)
                                nc.tensor.matmul(
                                    aggp[:, h * 128 : (h + 1) * 128],
                                    lhsT=g[:, h * 128 : (h + 1) * 128],
                                    rhs=Sq[q][:, j * 128 : (j + 1) * 128],
                                    start=(jglob == 0),
                                    stop=(jglob == nch - 1),
                                )
                                jglob += 1
                    agg2s = pool.tile([128, C1], BF16, tag="agg2s")
                    nc.vector.tensor_tensor(
                        agg2s[:],
                        aggp[:],
                        dcb2l2[:, w * 256 : (w + 1) * 256],
                        op=AL.mult,
                    )
                    h2p = psum2.tile([128, C2], F32, tag="l2p")
                    for m in range(2):
                        msl = slice(m * 128, (m + 1) * 128)
                        for k in range(2):
                            nc.tensor.matmul(
                                h2p[:, msl],
                                lhsT=w2r[
                                    :, k * 256 + m * 128 : k * 256 + (m + 1) * 128
                                ],
                                rhs=agg2s[:, k * 128 : (k + 1) * 128],
                                start=(k == 0),
                                stop=False,
                            )
                        nc.tensor.matmul(
                            h2p[:, msl],
                            lhsT=b2r[0:1, msl],
                            rhs=d2row[0:1, w * 128 : (w + 1) * 128],
                            start=False,
                            stop=True,
                        )
                    h2s = pool.tile([128, C2], BF16, tag="h2s")
                    nc.scalar.activation(h2s[:], h2p[:], ACT.Lrelu, alpha=0.01)
                    zp = psum2.tile([128, C3], F32, tag="zo")
                    nc.tensor.matmul(
                        zp[:, 0:C3],
                        lhsT=h2s[:, 0:128],
                        rhs=w3r[:, 0:C3],
                        start=True,
                        stop=False,
                    )
                    nc.tensor.matmul(
                        zp[:, 0:C3],
                        lhsT=h2s[:, 128:256],
                        rhs=w3r[:, C3 : 2 * C3],
                        start=False,
                        stop=True,
                    )
                    zt = pool.tile([128, C3], BF16, tag=f"zt{w}")
                    nc.vector.tensor_copy(zt[:], zp[:, 0:C3])
                    zts.append(zt)

                # ================= layer 3 =================
                # aggregate on the z-owner side via per-graph count matrices,
                # then one tiny f32 AllReduce; no z table, no gather.
                l3p = psum2.tile([128, C3], F32, tag="zo")
                for w in range(W2C):
                    nc.tensor.matmul(
                        l3p[:],
                        lhsT=s3c[:, w * 128 : (w + 1) * 128],
                        rhs=zts[w][:],
                        start=(w == 0),
                        stop=(w == W2C - 1),
                    )
                l3s = pool.tile([128, C3], F32, tag="l3s")
                nc.vector.tensor_copy(l3s[:], l3p[:])
                nc.sync.dma_start(out=ar_in[:, :], in_=l3s[:])
                nc.gpsimd.collective_compute(
                    "AllReduce",
                    AL.add,
                    replica_groups=rg,
                    ins=[ar_in.opt()],
                    outs=[ar_out.opt()],
                )
                ars = pool.tile([128, C3], F32, tag="ars")
                nc.sync.dma_start(out=ars[:, :], in_=ar_out[:, :])
                outt = pool.tile([128, C3], F32, tag="outt")
                nc.scalar.activation(
                    outt[:], ars[:], ACT.Copy, scale=disf[:, 0:1]
                )
                nc.vector.tensor_tensor(outt[:], outt[:], b3p[:], op=AL.add)
                nc.sync.dma_start(out=out_d[:, :], in_=outt[:])

    if compile_:
        nc.compile()
    return nc


# ---------------------------------------------------------------------------
# Entry point
# ---------------------------------------------------------------------------

_cache = {}


def _prepare(inputs):
    in_maps, meta = host_prep(**inputs)
    key = (
        meta["N"],
        meta["W1C"],
        meta["W2C"],
        meta["bounds"],
        meta["P1"],
        meta["P2Q"],
    )
    if key not in _cache:
        _cache[key] = build_program(meta)
    return _cache[key], in_maps, meta


def assemble_output(results, meta):
    G = meta["G"]
    return results[0]["out"][:G, :C3].astype(np.float32)


def kernel(**inputs):
    nc, in_maps, meta = _prepare(inputs)
    res = run_bass_kernel_spmd(nc, in_maps, core_ids=list(range(N_CORES)))
    return assemble_output(res.results, meta)


if __name__ == "__main__":
    rng = np.random.default_rng(0)
    N, E, G = 20000, 320000, 100
    inputs = dict(
        x=rng.standard_normal((N, 128), dtype=np.float32),
        src=rng.integers(0, N, E).astype(np.int32),
        dst=rng.integers(0, N, E).astype(np.int32),
        batch=(np.arange(N) // (N // G)).astype(np.int32),
        W1=rng.standard_normal((128, 256), dtype=np.float32) / 11.3,
        b1=rng.standard_normal(256).astype(np.float32) * 0.01,
        W2=rng.standard_normal((256, 256), dtype=np.float32) / 16.0,
        b2=rng.standard_normal(256).astype(np.float32) * 0.01,
        W3=rng.standard_normal((256, 32), dtype=np.float32) / 16.0,
        b3=rng.standard_normal(32).astype(np.float32) * 0.01,
        n_graphs=G,
    )
    out = kernel(**inputs)
    print("out", out.shape, out.dtype, float(np.abs(out).max()))
